# revision 43
# baseline (speedup 1.0000x reference)
"""Trainium2 Bass kernel for nn_CapsuleLayer: 2x2 conv (128->1024ch) + dynamic routing.

Strategy (data-parallel over batch, 4 samples per core on 8 cores):
  - Conv as 4 accumulated bf16 matmuls per tile: K=in_channels(128) on partitions.
    The stationary operand must be a single contiguous free dim, so each tile
    covers a contiguous 127-wide window of x = two conv-output rows plus one
    fake position (index 63, the w=63 wrap); fake positions get their routing
    weights zeroed, so they never contribute.
  - ut holds the UNBIASED conv output u^T[n, oc] in bf16 (plain DVE copy from
    PSUM, 2x mode); the conv bias is applied on the transposed copies into
    u_A[oc, n] via ACT Identity with a per-partition bias vector (free), and
    for the o-einsum (which streams unbiased ut) via an exact fixup
    o += bias[i,jk] * s_c[i] with s_c[i] = sum_n c[n,i] (one [1,512] matmul
    per softmax group against a ones column).
  - PE-transposes of ut into u_A are interleaved with the conv matmul tiles so
    the PE HAM clock gate stays warm (transpose-mode activity alone lets the
    PE fall back to 1.2 GHz). Row-sums for routing iteration 0 ride along on
    the transpose copies via accum_out (ACT for 8-chunk batches, DVE
    tensor_scalar for the tail pairs), with fake-column corrections
    subtracted afterwards.
  - Routing iterations as matmuls: the b-update uses u_A chunks as stationary
    weights, producing b^T[n, i] directly (softmax-friendly layout); softmax is
    ACT exp + DVE reduce/reciprocal; the o-einsum streams u^T with c^T
    stationary, COLUMN-TILED 4x: chunk t goes to PE column-group t%4 and PSUM
    partition strip 32*(t%4), so four chunks stream concurrently; diagonal
    extraction via constant mask + strided reduce + partition-strip folds.
  - All rsqrt/sqrt go through a DVE bit-trick (fast-inverse-sqrt + Newton) and
    ACT Square-with-accum, so the only ACT table functions used are
    {Copy, Identity, Exp, Square} (one table set, no 1.28us ACT table
    reloads on the serial inter-iteration chains).
  - The next sample's conv is emitted before the current sample's squash tail
    so the PE rolls straight into it.
"""
import os
import sys

sys.path.insert(0, "/opt/trn_rl_repo")

import numpy as np
import ml_dtypes

B, IN_C, H, W_SP = 32, 128, 64, 64
NUM_CAPS, D0, D1 = 32, 4, 8
OUT_C = NUM_CAPS * D0 * D1  # 1024
N_CORES = 8
SPC = B // N_CORES          # samples per core = 4
HO = WO = 63                # conv output spatial
NCH = 32                    # n-chunks per sample (2 output rows each; last = 1)
NV = 127                    # rows per chunk: 63 + fake + 63 (last: 63 + 64 junk)
XW = H * W_SP + 128         # x padded so the last window stays in bounds
GC = 16                     # chunks per softmax group
NG = NCH // GC              # groups per pass

# cons_f column offsets
CF_MASKBD = 0
CF_MASK2 = 256
CF_RMA = 288
CF_RMB = 304
CF_BIAST = 320
CF_BIASR = 328
CF_MAGIC = 360
CF_MDIAG = 361
CF_COLS = 361 + 1024

RSQRT_MAGIC = float(0x5F3759DF)  # fast-inverse-sqrt magic, as a float value
# cons_h column offsets
CH_EYE = 0
CH_IREP = 128
CH_RHS5 = 256
CH_ONES = 288
CH_COLS = 289

# bench: repeat the whole body KREPS times on-device (timing only)
KREPS = int(os.environ.get("KREPS", "1"))

_BUILT = {}


def _build_nc():
    import concourse.bacc as bacc
    import concourse.mybir as mybir
    import concourse.tile as tile

    F32 = mybir.dt.float32
    F32R = mybir.dt.float32r
    BF16 = mybir.dt.bfloat16

    nc = bacc.Bacc("TRN2")

    xs = nc.dram_tensor("xs", [SPC, 128, XW], BF16, kind="ExternalInput")
    wt = nc.dram_tensor("wt", [128, 4 * OUT_C], BF16, kind="ExternalInput")
    cons_f = nc.dram_tensor("cons_f", [128, CF_COLS], F32, kind="ExternalInput")
    cons_h = nc.dram_tensor("cons_h", [128, CH_COLS], BF16, kind="ExternalInput")
    out_d = nc.dram_tensor("out", [SPC, 32, 32], F32, kind="ExternalOutput")

    with nc.allow_low_precision("u is stored in bf16 by design"):
        with tile.TileContext(nc) as tc:
            _emit(nc, tc, mybir, F32, F32R, BF16,
                  xs, wt, cons_f, cons_h, out_d)
    nc.compile()
    return nc


def _emit(nc, tc, mybir, F32, F32R, BF16, xs, wt, cons_f, cons_h, out_d):
    from contextlib import ExitStack

    AF = mybir.ActivationFunctionType
    ALU = mybir.AluOpType
    AX = mybir.AxisListType

    with ExitStack() as ctx:
        const = ctx.enter_context(tc.tile_pool(name="const", bufs=1))
        big = ctx.enter_context(tc.tile_pool(name="big", bufs=1))
        xpool = ctx.enter_context(tc.tile_pool(name="xp", bufs=2))
        work = ctx.enter_context(tc.tile_pool(name="work", bufs=2))
        ps = ctx.enter_context(tc.tile_pool(name="ps", bufs=1, space="PSUM"))

        wt_t = const.tile([128, 4 * OUT_C], BF16)
        cf_t = const.tile([128, CF_COLS], F32)
        ch_t = const.tile([128, CH_COLS], BF16)
        nc.gpsimd.dma_start(wt_t[:], wt[:, :])
        nc.gpsimd.dma_start(cf_t[:], cons_f[:, :])
        nc.gpsimd.dma_start(ch_t[:], cons_h[:, :])
        maskbd = cf_t[:, CF_MASKBD:CF_MASKBD + 256]
        mask2 = cf_t[:, CF_MASK2:CF_MASK2 + 32]
        rmA = cf_t[:, CF_RMA:CF_RMA + GC]
        rmB = cf_t[:, CF_RMB:CF_RMB + GC]
        biasT = cf_t[:, CF_BIAST:CF_BIAST + 8]
        bias_resh = cf_t[0:32, CF_BIASR:CF_BIASR + 32]
        magic = cf_t[0:32, CF_MAGIC:CF_MAGIC + 1]
        maskdiag = cf_t[:, CF_MDIAG:CF_MDIAG + 1024]
        eye = ch_t[:, CH_EYE:CH_EYE + 128]
        irep = ch_t[0:32, CH_IREP:CH_IREP + 128]
        rhs5 = ch_t[:, CH_RHS5:CH_RHS5 + 32]
        ones_h = ch_t[:, CH_ONES:CH_ONES + 1]

        ut = big.tile([128, NCH, OUT_C], BF16)        # u^T: [n-part, chunk, oc]
        ua = big.tile([128, 8, NCH, 128], BF16)       # u_A: [oc-part, g, chunk, n]
        # column 127 of each chunk is never written (transposes fill 0:127);
        # it only feeds b-mm output row 127, which is never read — zero it
        # once so nothing uninitialized is ever loaded
        nc.vector.memset(ua[:, :, :, 127], 0.0)
        out_sb = big.tile([32, SPC, 32], F32)

        # ---- transpose batches: ut chunks -> ua, bias added, o0 accum ----
        def t_batch8(g, b4, o0p):
            ptr = ps.tile([128, 8, 128], BF16, tag="pmix", bufs=2)
            for j in range(8):
                t = 8 * b4 + j
                nc.tensor.transpose(ptr[:, j, 0:NV],
                                    ut[0:NV, t, 128 * g:128 * (g + 1)],
                                    eye[0:NV, 0:NV])
            nc.scalar.activation(ua[:, g, 8 * b4:8 * b4 + 8, 0:NV],
                                 ptr[:, :, 0:NV], AF.Identity,
                                 bias=biasT[:, g:g + 1],
                                 accum_out=o0p[:, g, b4:b4 + 1])

        def t_pair(g, k, o0p):
            ptr = ps.tile([128, 2, 128], BF16, tag="pmix", bufs=2)
            for j in range(2):
                t = 24 + 2 * k + j
                nc.tensor.transpose(ptr[:, j, 0:NV],
                                    ut[0:NV, t, 128 * g:128 * (g + 1)],
                                    eye[0:NV, 0:NV])
            nc.vector.tensor_scalar(ua[:, g, 24 + 2 * k:26 + 2 * k, 0:NV],
                                    ptr[:, :, 0:NV], biasT[:, g:g + 1], None,
                                    op0=ALU.add, op1=ALU.add,
                                    accum_out=o0p[:, g, 3 + k:4 + k])

        x_tiles = {}

        def conv_phase(s, o0p):
            x_t = xpool.tile([128, XW], BF16)
            x_tiles[s] = x_t
            # split the load so tile 0's window arrives quickly
            nc.gpsimd.dma_start(x_t[:, 0:1152], xs[s, :, 0:1152])
            nc.gpsimd.dma_start(x_t[:, 1152:XW], xs[s, :, 1152:XW])
            for t in range(NCH):
                pc = ps.tile([128, 1024], F32, tag="pA", bufs=2)
                for h in range(2):
                    for kpos in range(4):
                        kh, kw = kpos // 2, kpos % 2
                        off = (2 * t + kh) * W_SP + kw
                        # 128-wide stationary (row 127 of the psum output is
                        # junk, never copied out) so FWL engages — a 127-col
                        # weight load runs at half the rate
                        nc.tensor.matmul(
                            pc[:, 512 * h:512 * (h + 1)],
                            x_t[:, off:off + 128],
                            wt_t[:, kpos * OUT_C + 512 * h: kpos * OUT_C + 512 * (h + 1)],
                            start=(kpos == 0), stop=(kpos == 3))
                # the fp32->bf16 cast runs at 1 elem/cycle on either engine;
                # put it on ACT for tiles where ACT has no transpose-batch
                # copy, on DVE otherwise
                if t < 8 or 12 <= t < 16 or 20 <= t < 24:
                    nc.scalar.copy(ut[0:NV, t, :], pc[0:NV, :])
                else:
                    nc.vector.tensor_copy(ut[0:NV, t, :], pc[0:NV, :])
                # interleave transposes of finished chunk-blocks with the conv
                # matmul stream so the PE HAM clock gate never sees a long
                # REGULAR-matmul-free window
                if 8 <= t <= 11:
                    t_batch8(2 * (t - 8), 0, o0p)
                    t_batch8(2 * (t - 8) + 1, 0, o0p)
                elif 16 <= t <= 19:
                    t_batch8(2 * (t - 16), 1, o0p)
                    t_batch8(2 * (t - 16) + 1, 1, o0p)
                elif 24 <= t <= 27:
                    t_batch8(2 * (t - 24), 2, o0p)
                    t_batch8(2 * (t - 24) + 1, 2, o0p)
                if t in (25, 26, 27, 28, 29, 30):
                    k = (t - 25) // 2
                    gs = range(0, 4) if (t - 25) % 2 == 0 else range(4, 8)
                    for g in gs:
                        t_pair(g, k, o0p)
            for g in range(8):
                t_pair(g, 3, o0p)  # chunks 30,31: small tail after the conv

        I32 = mybir.dt.int32

        def norm_stats(o_cur, tagp):
            """ssq[i] = sum_jk o^2 via one ACT Square with accum."""
            sq = work.tile([32, 32], F32, tag="t32")
            ssq = work.tile([32, 1], F32, tag="s1" + tagp)
            nc.scalar.activation(sq[:], o_cur[:], AF.Square, accum_out=ssq[:])
            return ssq

        def rsqrt(ssq, pref, steps=2):
            """rsqrt on the DVE only (bit trick + 2 Newton steps) so the ACT
            engine never needs the Ln table (keeps one ACT table set loaded
            and the inter-iteration serial chain short)."""
            t = work.tile([32, 1], I32, tag=pref + "t")
            nc.vector.tensor_scalar(t[:], ssq[:].bitcast(I32), 1, None,
                                    op0=ALU.logical_shift_right)
            y0i = work.tile([32, 1], I32, tag=pref + "y0")
            nc.vector.scalar_tensor_tensor(y0i[:], t[:], -1.0, magic,
                                           op0=ALU.mult, op1=ALU.add)
            ssqh = work.tile([32, 1], F32, tag=pref + "sh")
            nc.vector.tensor_scalar_mul(ssqh[:], ssq[:], 0.5)
            y = y0i[:].bitcast(F32)
            for step in range(steps):
                h = work.tile([32, 1], F32, tag=pref + "h%d" % step)
                nc.vector.scalar_tensor_tensor(h[:], y, ssqh[:], y,
                                               op0=ALU.mult, op1=ALU.mult)
                t2 = work.tile([32, 1], F32, tag=pref + "t2%d" % step)
                nc.vector.tensor_scalar(t2[:], h[:], 1.5, None,
                                        op0=ALU.subtract)
                yn = work.tile([32, 1], F32, tag=pref + "y%d" % step)
                nc.vector.scalar_tensor_tensor(yn[:], t2[:], -1.0, y,
                                               op0=ALU.mult, op1=ALU.mult)
                y = yn[:]
            return y

        def routing(s, o0p, o0p_next):
            # ---- o0 (uniform softmax iteration; global scale is irrelevant
            # because iteration 0 normalizes o) ----
            o0ch = work.tile([128, 8], F32, tag="o0ch")
            nc.vector.tensor_reduce(o0ch[:], o0p[:], axis=AX.X, op=ALU.add)
            f1 = work.tile([128, 8], F32, tag="f1")
            nc.vector.tensor_reduce(f1[:], ua[:, :, :, 63], axis=AX.X, op=ALU.add)
            f2 = work.tile([128, 8], F32, tag="f2")
            nc.vector.tensor_reduce(f2[:], ua[:, :, NCH - 1, 64:NV],
                                    axis=AX.X, op=ALU.add)
            nc.vector.tensor_sub(o0ch[:], o0ch[:], f1[:])
            nc.vector.tensor_sub(o0ch[:], o0ch[:], f2[:])
            # tiny REGULAR matmul keyed on the serial chain: keeps the PE HAM
            # window non-idle through the o0 computation (output unused)
            dj = ps.tile([32, 8], F32, tag="pA", bufs=2)
            nc.tensor.matmul(dj[:], mask2, o0ch[:], start=True, stop=True,
                             skip_group_check=True)
            lhsT5 = work.tile([128, 32], BF16, tag="lhsT5")
            nc.vector.tensor_tensor(
                lhsT5[:].rearrange("p (g q) -> p g q", q=4),
                o0ch[:].unsqueeze(2).broadcast_to([128, 8, 4]),
                mask2.rearrange("p (g q) -> p g q", q=4),
                op=ALU.mult)
            o_ps = ps.tile([32, 32], F32, tag="pA", bufs=2)
            nc.tensor.matmul(o_ps[:], lhsT5[:], rhs5, start=True, stop=True)
            o0_sb = work.tile([32, 32], F32, tag="onx")
            nc.vector.tensor_copy(o0_sb[:], o_ps[:])
            o_cur = o0_sb  # [32 i, 32 jk] f32

            # ---- routing iterations ----
            for it in range(2):
                # normalize o -> o' (bf16); one Newton step (~0.2% worst-case
                # on the normalization scale) keeps the serial chain short
                ssq = norm_stats(o_cur, "n")
                rn = rsqrt(ssq, "n", steps=1)
                o_n = work.tile([32, 32], BF16, tag="on")
                nc.scalar.activation(o_n[:], o_cur[:], AF.Copy, scale=rn)

                # ObdT: transpose o', strip-replicate via matmul, mask (one TT)
                oT_ps = ps.tile([32, 32], BF16, tag="pmix", bufs=2)
                nc.tensor.transpose(oT_ps[:], o_n[:], eye[0:32, 0:32])
                oT_sb = work.tile([32, 32], BF16, tag="oT")
                nc.vector.tensor_copy(oT_sb[:], oT_ps[:])
                s_ps = ps.tile([128, 32], F32, tag="pA", bufs=2)
                nc.tensor.matmul(s_ps[:], irep, oT_sb[:], start=True, stop=True)
                obdt = work.tile([128, 8, 32], BF16, tag="obdt")
                nc.vector.tensor_tensor(
                    obdt[:],
                    s_ps[:].unsqueeze(1).broadcast_to([128, 8, 32]),
                    maskbd.rearrange("p (g i) -> p g i", i=32),
                    op=ALU.mult)

                # fused pass over NG groups: b-mm -> softmax -> o-mm (4x
                # column-tiled: chunk t streams on PE column-group t%4 into
                # PSUM partition strip 32*(t%4))
                po = ps.tile([128, 1024], F32, tag="po", bufs=1)
                cts = [None] * NG
                scrows = [None] * NG

                def o_mms(grp):
                    for j in range(GC):
                        t = GC * grp + j
                        q = t % 4
                        for h in range(2):
                            nc.tensor.matmul(
                                po[32 * q:32 * (q + 1), 512 * h:512 * (h + 1)],
                                cts[grp][0:NV, j, :],
                                ut[0:NV, t, 512 * h:512 * (h + 1)],
                                start=(t == q), stop=(t == 28 + q),
                                tile_position=(0, 32 * q),
                                skip_group_check=True)

                def sc_emit(grp):
                    # s_c partial: sum_n c[n, i] for this group's chunks.
                    # Emitted only at points where the PE queue ahead of it
                    # has ready work, so the wait on the softmax DVE chain
                    # never stalls the PE.
                    sc_ps = ps.tile([1, GC, 32], F32, tag="pA", bufs=2)
                    nc.tensor.matmul(sc_ps[:], ones_h[0:NV, :], cts[grp][0:NV],
                                     start=True, stop=True)
                    scrow = work.tile([1, 32], F32, tag="scr%d" % grp)
                    nc.vector.tensor_reduce(
                        scrow[:], sc_ps[:].rearrange("p j i -> p i j"),
                        axis=AX.X, op=ALU.add)
                    scrows[grp] = scrow

                for grp in range(NG):
                    pb_t = ps.tile([128, GC, 32], F32, tag="pmix", bufs=2)
                    for j in range(GC):
                        t = GC * grp + j
                        for g in range(8):
                            nc.tensor.matmul(pb_t[:, j, :], ua[:, g, t, :],
                                             obdt[:, g, :],
                                             start=(g == 0), stop=(g == 7))
                    if grp == 1:
                        sc_emit(0)
                    e_g = work.tile([128, GC, 32], F32, tag="eg")
                    nc.scalar.activation(e_g[0:NV], pb_t[0:NV], AF.Exp)
                    z_g = work.tile([128, GC], F32, tag="zg")
                    nc.vector.tensor_reduce(z_g[0:NV], e_g[0:NV], axis=AX.X,
                                            op=ALU.add)
                    zi_g = work.tile([128, GC], F32, tag="zig")
                    nc.vector.reciprocal(zi_g[0:NV], z_g[0:NV])
                    # zero the fake rows' routing weights: row 63 of every
                    # chunk; rows 64:127 of the last chunk (rmB, last col)
                    zi2 = work.tile([128, GC], F32, tag="zi2")
                    nc.vector.tensor_tensor(zi2[0:NV], zi_g[0:NV],
                                            (rmB if grp == NG - 1 else rmA)[0:NV, :],
                                            op=ALU.mult)
                    ct = work.tile([128, GC, 32], BF16, tag="ct")
                    nc.vector.tensor_tensor(
                        ct[0:NV], e_g[0:NV],
                        zi2[0:NV].unsqueeze(2).broadcast_to([NV, GC, 32]),
                        op=ALU.mult)
                    cts[grp] = ct
                    if grp > 0:
                        o_mms(grp - 1)
                        sc_emit(grp)

                # s_c total -> column vector via tiny PE transpose; emitted
                # before the last o-mm block so it runs under the o-mm stream
                # instead of inside the serial iteration boundary
                sc_h = work.tile([1, 32], BF16, tag="sch")
                nc.vector.tensor_tensor(sc_h[:], scrows[0][:], scrows[1][:],
                                        op=ALU.add)
                sct_ps = ps.tile([32, 1], BF16, tag="pmix", bufs=2)
                nc.tensor.transpose(sct_ps[:], sc_h[:], eye[0:1, 0:1])
                sct = work.tile([32, 1], F32, tag="sct")
                nc.vector.tensor_copy(sct[:], sct_ps[:])

                o_mms(NG - 1)

                if it == 1 and s + 1 < SPC:
                    conv_phase(s + 1, o0p_next)  # overlap next conv

                # diagonal extraction on the 128-partition strip layout
                tmpd = work.tile([128, 1024], F32, tag="tmpd")
                nc.vector.tensor_tensor(tmpd[:], po[:], maskdiag,
                                        op=ALU.mult)
                red = work.tile([128, 32], BF16, tag="red")
                nc.vector.tensor_reduce(red[:],
                                        tmpd[:].rearrange("p (i k) -> p k i", k=32),
                                        axis=AX.X, op=ALU.add)
                # fold the 4 partition strips: rhs5[p, i] = (p%32 == i) is
                # exactly the needed selection matrix
                fold_ps = ps.tile([32, 32], F32, tag="pA", bufs=2)
                nc.tensor.matmul(fold_ps[:], rhs5, red[:], start=True, stop=True)
                # bias fixup: o = fold + bias_resh * s_c[i]
                o_nx = work.tile([32, 32], F32, tag="onx")
                nc.vector.scalar_tensor_tensor(o_nx[:], bias_resh, sct[:],
                                               fold_ps[:], op0=ALU.mult,
                                               op1=ALU.add)
                o_cur = o_nx

            # ---- squash ----
            ssq = norm_stats(o_cur, "q")
            rq = rsqrt(ssq, "q")
            sq_s = work.tile([32, 1], F32, tag="s2q")
            nc.vector.tensor_tensor(sq_s[:], ssq[:], rq, op=ALU.mult)
            d2 = work.tile([32, 1], F32, tag="s4")
            nc.vector.tensor_scalar_add(d2[:], sq_s[:], 1e-6)
            denom = work.tile([32, 1], F32, tag="s5")
            nc.vector.scalar_tensor_tensor(denom[:], ssq[:], 1.0, d2[:],
                                           op0=ALU.add, op1=ALU.mult)
            r = work.tile([32, 1], F32, tag="s6")
            nc.vector.reciprocal(r[:], denom[:])
            f = work.tile([32, 1], F32, tag="s7")
            nc.vector.tensor_tensor(f[:], ssq[:], r[:], op=ALU.mult)
            nc.scalar.activation(out_sb[:, s, :], o_cur[:], AF.Copy, scale=f[:])

        if KREPS > 1:
            rep_ctx = tc.For_i(0, KREPS, 1)
            rep_ctx.__enter__()

        o0p_cur = work.tile([128, 8, 7], F32, tag="o0p")
        conv_phase(0, o0p_cur)
        for s in range(SPC):
            o0p_next = (work.tile([128, 8, 7], F32, tag="o0p", name="o0p_next")
                        if s + 1 < SPC else None)
            routing(s, o0p_cur, o0p_next)
            o0p_cur = o0p_next

        if KREPS > 1:
            rep_ctx.__exit__(None, None, None)

        nc.gpsimd.dma_start(out_d.rearrange("s i j -> i s j"), out_sb[:])


def _consts():
    p = np.arange(128)
    i = np.arange(32)
    g = np.arange(8)
    maskbd = (i[None, None, :] == 4 * g[None, :, None] + p[:, None, None] // 32)
    mask2 = (p[:, None] // 32 == i[None, :] % 4)
    maskdiag128 = (np.arange(OUT_C)[None, :] // 32 == p[:, None] % 32)
    rmA = np.ones((128, GC), np.float32)
    rmA[63] = 0.0
    rmB = rmA.copy()
    rmB[64:, GC - 1] = 0.0

    eye = np.eye(128, dtype=np.float32)
    irep_p = np.zeros((128, 128), np.float32)
    irep_p[0:32] = (np.arange(32)[:, None] == p[None, :] % 32)   # [q, p]
    rhs5 = (p[:, None] % 32 == i[None, :]).astype(np.float32)     # [p, jk]
    ones_col = np.ones((128, 1), np.float32)
    cons_h = np.concatenate([eye, irep_p, rhs5, ones_col],
                            axis=1).astype(ml_dtypes.bfloat16)
    return maskbd, mask2, maskdiag128, rmA, rmB, cons_h


def make_inputs(W, b_conv):
    """Host-side constant packing shared by kernel() and sim_check."""
    # Wt[c, kpos*1024 + oc] = W[oc, c, kh, kw]
    wt = np.ascontiguousarray(
        W.reshape(OUT_C, IN_C, 4).transpose(1, 2, 0).reshape(IN_C, 4 * OUT_C)
    ).astype(ml_dtypes.bfloat16)
    maskbd, mask2, maskdiag128, rmA, rmB, cons_h = _consts()
    biasT = np.ascontiguousarray(
        b_conv.reshape(8, 128).T).astype(np.float32)              # [p, g]
    bias_resh_p = np.zeros((128, 32), np.float32)
    bias_resh_p[0:32] = b_conv.reshape(32, 32)
    magic_col = np.full((128, 1), RSQRT_MAGIC, np.float32)
    cons_f = np.concatenate(
        [maskbd.reshape(128, 256).astype(np.float32),
         mask2.astype(np.float32), rmA, rmB, biasT, bias_resh_p, magic_col,
         maskdiag128.astype(np.float32)], axis=1)
    assert cons_f.shape[1] == CF_COLS
    return wt, cons_f, cons_h


def pack_x(x):
    """[B, 128, H*W] f32 -> [B, 128, XW] bf16 zero-padded."""
    xf = np.zeros((B, 128, XW), np.float32)
    xf[:, :, :H * W_SP] = x.reshape(B, 128, H * W_SP)
    return xf.astype(ml_dtypes.bfloat16)


def kernel(x, W, b_conv):
    from concourse.bass_utils import run_bass_kernel_spmd

    x = np.asarray(x, dtype=np.float32)
    W = np.asarray(W, dtype=np.float32)
    b_conv = np.asarray(b_conv, dtype=np.float32)
    wt, cons_f, cons_h = make_inputs(W, b_conv)

    if "nc" not in _BUILT:
        _BUILT["nc"] = _build_nc()
    nc = _BUILT["nc"]

    xp = pack_x(x)

    in_maps = []
    for c in range(N_CORES):
        in_maps.append({"xs": np.ascontiguousarray(xp[c * SPC:(c + 1) * SPC]),
                        "wt": wt, "cons_f": cons_f, "cons_h": cons_h})

    global _last_in_maps
    _last_in_maps = in_maps
    res = run_bass_kernel_spmd(nc, in_maps, core_ids=list(range(N_CORES)))
    out = np.concatenate([r["out"] for r in res.results], axis=0)
    return out.astype(np.float32)


_last_in_maps = None
